# revision 32
# baseline (speedup 1.0000x reference)
"""DecoderTreeRNN Trainium2 kernel.

Computes: h0 = relu(encoding); expand a depth-`depth` binary tree with two
zero-input GRU cells (left/right); project every leaf hidden state with W_out
and take log_softmax over the vocab.

Strategy: pure data parallel over 8 NeuronCores (batch sharded), GRU weights
and the output projection replicated.  On-core layout is transposed
([hidden-chunk on partitions, tokens on the free dim]) so all matmuls
contract over partitions and the softmax reduction runs along the free dim.
"""

import os
import sys
from contextlib import ExitStack

import numpy as np

for _p in ("/opt/trn_rl_repo", "/root/.axon_site/_ro/trn_rl_repo"):
    if os.path.isdir(_p) and _p not in sys.path:
        sys.path.insert(0, _p)

import ml_dtypes

N_CORES = 8
P = 128
TTILE = 512  # token tile for GRU matmuls (max fp32 moving free dim)
NBF = 512  # fp32 elements per PSUM bank
VGW = 4 * NBF  # vocab group width (4 PSUM banks)
SPLIT_FRAC = 0.45  # fraction of final log_softmax subtract done on ScalarE

# Set by test harness to capture a profile on the next kernel() call.
TRACE = False
# CoreSim cannot interpret strided-partition DMA views; test_sim sets this
# to fall back to dense per-leaf stores (HW always uses the strided path).
SIM_SAFE_DMA = False
LAST_EXEC_NS = None
LAST_RESULTS = None

_COMPILE_CACHE = {}


def _bitrev(x, bits):
    r = 0
    for _ in range(bits):
        r = (r << 1) | (x & 1)
        x >>= 1
    return r


def _numpy_reference(encoding, W_hh_l, b_ih_l, b_hh_l, W_hh_r, b_ih_r, b_hh_r,
                     W_out, b_out, depth):
    def gru(h, W, b_ih, b_hh):
        Hd = h.shape[-1]
        gh = h @ W.T + b_hh
        r = 1.0 / (1.0 + np.exp(-(b_ih[:Hd] + gh[..., :Hd])))
        z = 1.0 / (1.0 + np.exp(-(b_ih[Hd:2 * Hd] + gh[..., Hd:2 * Hd])))
        n = np.tanh(b_ih[2 * Hd:] + r * gh[..., 2 * Hd:])
        return (1.0 - z) * n + z * h

    h = np.maximum(encoding, 0.0)[:, None, :]
    for _ in range(depth):
        left = gru(h, W_hh_l, b_ih_l, b_hh_l)
        right = gru(h, W_hh_r, b_ih_r, b_hh_r)
        h = np.stack([left, right], axis=2).reshape(h.shape[0], -1, h.shape[-1])
    logits = h @ W_out.T + b_out
    m = logits.max(axis=-1, keepdims=True)
    e = np.exp(logits - m)
    return (logits - m) - np.log(e.sum(axis=-1, keepdims=True))


def _patch_act_tables(bacc, mybir):
    """Constrain the ACT table-set chooser so the GRU phase and the
    projection phase each stick to ONE set (2 loads total instead of 2
    per token chunk).  Only the chooser's view is filtered; the runtime
    tables are the real (full) sets, so execution is unchanged."""
    from concourse import hw_specs
    AF = mybir.ActivationFunctionType
    orig = hw_specs.get_activation_tables
    if getattr(bacc.get_activation_tables, "_treernn_patch", False):
        return
    keep = {
        "sigmoid_and_others": {AF.Sigmoid, AF.Tanh, AF.Relu},
        "natural_log_exp_and_others": {AF.Exp, AF.Ln, AF.Identity, AF.Copy},
    }
    controlled = set().union(*keep.values())

    def patched(arch):
        tabs = {k: set(v) for k, v in orig(arch).items()}
        for name, s in tabs.items():
            s -= controlled
            s |= keep.get(name, set())
        return tabs

    patched._treernn_patch = True
    bacc.get_activation_tables = patched


def _build(Bc, H, V, depth):
    """Build + compile the single-core SPMD program (identical on all cores)."""
    import concourse.bass as bass  # noqa: F401
    import concourse.tile as tile
    from concourse import bacc, mybir

    f32 = mybir.dt.float32
    bf16 = mybir.dt.bfloat16
    AF = mybir.ActivationFunctionType
    OP = mybir.AluOpType
    _patch_act_tables(bacc, mybir)

    KH = H // P
    H3 = 3 * H
    L = 1 << depth
    TOK = Bc * L
    NTC = (TOK + P - 1) // P
    # vocab tiles (one PSUM bank each) and groups of up to 4 tiles
    vts = [NBF] * (V // NBF) + ([V % NBF] if V % NBF else [])
    vgroups = []
    pos = 0
    while pos < V:
        w = min(VGW, V - pos)
        vgroups.append((pos, w))
        pos += w
    NVG = len(vgroups)
    split = min(V, max(0, int(V * SPLIT_FRAC)) & ~15)

    nc = bacc.Bacc("TRN2", target_bir_lowering=False, debug=False,
                   num_devices=N_CORES)

    enc_d = nc.dram_tensor("enc_t", [KH, P, Bc], f32, kind="ExternalInput").ap()
    whh_d = {s: nc.dram_tensor(f"whht_{s}", [KH, P, H3], bf16,
                               kind="ExternalInput").ap() for s in "lr"}
    # packed per-side biases: cols [0:2K]=sigmoid(r,z), [2K:3K]=tanh, [3K:4K]=n_hh
    bias_d = {s: nc.dram_tensor(f"bias_{s}", [P, 4 * KH], f32,
                                kind="ExternalInput").ap() for s in "lr"}
    wout_d = nc.dram_tensor("woutt", [KH, P, V], bf16, kind="ExternalInput").ap()
    bout_d = nc.dram_tensor("bout", [P, V], bf16, kind="ExternalInput").ap()
    out_d = nc.dram_tensor("out", [Bc, L, V], f32, kind="ExternalOutput").ap()

    # all vocab-group weight tiles are staged before the GRU phase: their
    # DMA (~10 MB) overlaps GRU compute and fits SBUF since the GRU state
    # went bf16
    n_wv_early = NVG

    with tile.TileContext(nc) as tc, ExitStack() as ctx:
        constp = ctx.enter_context(tc.tile_pool(name="const", bufs=1))
        ht2p = ctx.enter_context(tc.tile_pool(name="ht2", bufs=1))
        ht2 = ht2p.tile([P, KH, TOK], bf16)
        wvep = ctx.enter_context(tc.tile_pool(name="wout_early", bufs=1))
        bop = ctx.enter_context(tc.tile_pool(name="bout", bufs=1))

        bsig, btanh, bnhh = {}, {}, {}
        for s in "lr":
            bt = constp.tile([P, 4 * KH], f32, name=f"bias{s}")
            nc.sync.dma_start(out=bt, in_=bias_d[s])
            bsig[s] = bt[:, :2 * KH]
            btanh[s] = bt[:, 2 * KH:3 * KH]
            bnhh[s] = bt[:, 3 * KH:]

        def load_proj_consts():
            # issued AFTER the GRU weights so they don't head-block the
            # sync-engine DMA queue and delay the GRU start
            bout_sb = bop.tile([P, V], bf16)
            nc.sync.dma_start(out=bout_sb, in_=bout_d)
            wv = []
            for vg, (vs, vw) in enumerate(vgroups[:n_wv_early]):
                wt = wvep.tile([P, KH, vw], bf16, name=f"wv{vg}")
                for k in range(KH):
                    nc.sync.dma_start(out=wt[:, k, :],
                                      in_=wout_d[k, :, vs:vs + vw])
                wv.append(wt)
            return bout_sb, wv

        # ---------------- GRU tree expansion (fp32) ----------------
        with tc.tile_pool(name="gwhh", bufs=1) as gwp, \
             tc.tile_pool(name="gh", bufs=1) as ghp, \
             tc.tile_pool(name="gact", bufs=2) as gap, \
             tc.tile_pool(name="gactd", bufs=1) as gdp, \
             tc.tile_pool(name="gpsum", bufs=2, space="PSUM") as gpp:
            whh = {}
            for s in "lr":
                w = gwp.tile([P, KH, H3], bf16, name=f"whh{s}")
                for k in range(KH):
                    nc.sync.dma_start(out=w[:, k, :], in_=whh_d[s][k])
                whh[s] = w

            enc_sb = gdp.tile([P, KH, Bc], f32, name="enc_stage")
            for k in range(KH):
                nc.sync.dma_start(out=enc_sb[:, k, :], in_=enc_d[k])
            h_cur = ghp.tile([P, KH, Bc], bf16, name="h_l0")
            nc.scalar.activation(out=h_cur, in_=enc_sb, func=AF.Relu)

            bout_sb, wv = load_proj_consts()

            for lvl in range(depth):
                t = Bc << lvl
                h_nxt = ghp.tile([P, KH, 2 * t], bf16, name=f"h_l{lvl + 1}")
                for si, s in enumerate("lr"):
                    soff = si * t
                    for t0 in range(0, t, TTILE):
                        tt = min(TTILE, t - t0)
                        hs = h_cur[:, :, t0:t0 + tt]
                        r_sb = gap.tile([P, KH, TTILE], bf16, name="g_r")[:, :, :tt]
                        z_sb = gap.tile([P, KH, TTILE], bf16, name="g_z")[:, :, :tt]
                        n_sb = gap.tile([P, KH, TTILE], bf16, name="g_n")[:, :, :tt]
                        d_sb = gdp.tile([P, KH, TTILE], bf16, name="g_d")[:, :, :tt]
                        for gi in range(3):  # r, z, n
                            ps = gpp.tile([P, KH, TTILE], f32,
                                          name="g_ps")[:, :, :tt]
                            for gc in range(KH):
                                col = gi * H + gc * P
                                for k in range(KH):
                                    nc.tensor.matmul(
                                        ps[:, gc, :],
                                        lhsT=whh[s][:, k, col:col + P],
                                        rhs=hs[:, k, :],
                                        start=(k == 0), stop=(k == KH - 1))
                            if gi == 0:
                                for gc in range(KH):
                                    nc.scalar.activation(
                                        out=r_sb[:, gc, :], in_=ps[:, gc, :],
                                        func=AF.Sigmoid,
                                        bias=bsig[s][:, gc:gc + 1])
                            elif gi == 1:
                                for gc in range(KH):
                                    nc.scalar.activation(
                                        out=z_sb[:, gc, :], in_=ps[:, gc, :],
                                        func=AF.Sigmoid,
                                        bias=bsig[s][:, KH + gc:KH + gc + 1])
                            else:
                                for gc in range(KH):
                                    # n_pre = (gh_n + b_hh_n) * r
                                    nc.vector.scalar_tensor_tensor(
                                        out=n_sb[:, gc, :], in0=ps[:, gc, :],
                                        scalar=bnhh[s][:, gc:gc + 1],
                                        in1=r_sb[:, gc, :],
                                        op0=OP.add, op1=OP.mult)
                                for gc in range(KH):
                                    nc.scalar.activation(
                                        out=n_sb[:, gc, :], in_=n_sb[:, gc, :],
                                        func=AF.Tanh,
                                        bias=btanh[s][:, gc:gc + 1])
                        # h' = n + z * (h - n)
                        nc.vector.tensor_tensor(d_sb, hs, n_sb, OP.subtract)
                        nc.vector.tensor_tensor(d_sb, d_sb, z_sb, OP.mult)
                        nc.vector.tensor_tensor(
                            h_nxt[:, :, soff + t0:soff + t0 + tt],
                            d_sb, n_sb, OP.add)
                h_cur = h_nxt

            # in-order leaf layout: node blocks were built choice-LSB-first,
            # reference wants choice-MSB-first -> bit-reversal permute.
            # Within each 128-token chunk, tokens are INTERLEAVED as
            # p = b*NLL + ll so a per-leaf output DMA reads stride-NLL
            # partitions (touches all 16 SBUF port groups, not 4).
            NLL = max(1, min(P // Bc, L))  # leaves per token chunk
            for node in range(L):
                leaf = _bitrev(node, depth)
                tcp, ll = leaf // NLL, leaf % NLL
                base = tcp * P + ll
                nc.vector.tensor_copy(
                    out=ht2[:, :, base:base + (Bc - 1) * NLL + 1:NLL],
                    in_=h_cur[:, :, node * Bc:(node + 1) * Bc])

        # ---------------- output projection + log_softmax ----------------
        with tc.tile_pool(name="wout_late", bufs=1) as wvp, \
             tc.tile_pool(name="ypool", bufs=2) as yp, \
             tc.tile_pool(name="stat", bufs=2) as stp, \
             tc.tile_pool(name="escratch", bufs=2) as esp, \
             tc.tile_pool(name="ppsum", bufs=2, space="PSUM") as ppp:
            for vg, (vs, vw) in enumerate(vgroups[n_wv_early:], n_wv_early):
                wt = wvp.tile([P, KH, vw], bf16, name=f"wv{vg}")
                for k in range(KH):
                    nc.sync.dma_start(out=wt[:, k, :], in_=wout_d[k, :, vs:vs + vw])
                wv.append(wt)

            def out_dma(tci, pc, y, v0, v1):
                """Per-leaf stores; partitions are interleaved (p = b*NLL+ll)
                so each DMA's 32 source partitions stride across all 16 SBUF
                port groups and its rows spread over all 16 SDMA engines."""
                nll = pc // Bc
                if SIM_SAFE_DMA:
                    # dense-partition source; dst dims follow p = b*nll + ll
                    dst = out_d.rearrange("b (lc ll) v -> lc b ll v", ll=nll)
                    nc.sync.dma_start(out=dst[tci][:, :, v0:v1],
                                      in_=y[:, v0:v1])
                    return
                for ll in range(nll):
                    leaf = tci * nll + ll
                    nc.sync.dma_start(out=out_d[:, leaf, v0:v1],
                                      in_=y[ll:pc:nll, v0:v1])

            def emit_tail(st):
                """Softmax tail for a finished chunk: c = ln(sum), out -= c.
                High priority: its DMA frees the y buffer that gates the
                chunk-after-next, so it must not lose the scheduler race."""
                tci, pc, y, sums, cs = st
                nc.vector.tensor_reduce(out=cs[:, 0:1], in_=sums,
                                        axis=mybir.AxisListType.X, op=OP.add)
                nc.scalar.activation(out=cs[:, 0:1], in_=cs[:, 0:1],
                                     func=AF.Ln)
                nc.vector.tensor_scalar(out=cs[:, 1:2], in0=cs[:, 0:1],
                                        scalar1=-1.0, scalar2=None,
                                        op0=OP.mult)
                if split > 0:
                    nc.scalar.activation(out=y[:, :split], in_=y[:, :split],
                                         func=AF.Identity, bias=cs[:, 1:2])
                    out_dma(tci, pc, y, 0, split)
                if split < V:
                    # deprioritized: otherwise the static scheduler places
                    # this 3us op ahead of the next chunk's first bias-add in
                    # the VectorE FIFO and stalls the PSUM-slot recycle
                    with tc.high_priority(offset=-60):
                        nc.vector.tensor_scalar(out=y[:, split:],
                                                in0=y[:, split:],
                                                scalar1=cs[:, 0:1],
                                                scalar2=None,
                                                op0=OP.subtract)
                        out_dma(tci, pc, y, split, V)

            pending = None  # previous chunk's tail, pipelined one chunk late
            for tci in range(NTC):
                pc = min(P, TOK - tci * P)  # tokens in this chunk
                y = yp.tile([P, V], f32, name="y")[:pc]
                sums = stp.tile([P, NVG], f32, name="sums")[:pc]
                cs = stp.tile([P, 2], f32, name="cs")[:pc]
                for vg, (vs, vw) in enumerate(vgroups):
                    ps = ppp.tile([P, VGW], f32, name="p_vg")[:pc, :vw]
                    for vt0 in range(0, vw, NBF):
                        w = min(NBF, vw - vt0)
                        pslice = ps[:, vt0:vt0 + w]
                        for k in range(KH):
                            nc.tensor.matmul(
                                pslice,
                                lhsT=ht2[:, k, tci * P:tci * P + pc],
                                rhs=wv[vg][:, k, vt0:vt0 + w],
                                start=(k == 0), stop=(k == KH - 1))
                    nc.vector.tensor_tensor(
                        y[:, vs:vs + vw], ps, bout_sb[:pc, vs:vs + vw], OP.add)
                    # exp main output is discarded (into SBUF scratch, NOT the
                    # PSUM tile — writing PSUM would extend the slot lifetime
                    # and stall the matmuls two vocab-groups ahead);
                    # only the per-partition running sum matters
                    esc = esp.tile([P, VGW], bf16, name="e_sc")[:pc, :vw]
                    nc.scalar.activation(out=esc, in_=y[:, vs:vs + vw],
                                         func=AF.Exp,
                                         accum_out=sums[:, vg:vg + 1])
                    # previous chunk's softmax tail, issued after this
                    # chunk's first vocab-group so its DMA starts early but
                    # doesn't head-block this chunk's drain chain
                    if vg == 0 and pending is not None:
                        emit_tail(pending)
                pending = (tci, pc, y, sums, cs)
            emit_tail(pending)

    nc.compile()
    return nc


def _packed_bias(b_ih, b_hh, H, KH):
    """[P, 4*KH]: sigmoid biases (b_ih+b_hh for r,z), tanh bias (b_ih_n),
    and the pre-multiply n-gate bias (b_hh_n), per 128-row chunk."""
    P = 128
    sig = (b_ih + b_hh)[:2 * H].reshape(2 * KH, P).T
    tanh = b_ih[2 * H:].reshape(KH, P).T
    nhh = b_hh[2 * H:].reshape(KH, P).T
    return np.ascontiguousarray(np.concatenate([sig, tanh, nhh], axis=1))


def _get_compiled(Bc, H, V, depth):
    key = (Bc, H, V, depth)
    if key not in _COMPILE_CACHE:
        _COMPILE_CACHE[key] = _build(Bc, H, V, depth)
    return _COMPILE_CACHE[key]


def kernel(encoding, W_hh_l, b_ih_l, b_hh_l, W_hh_r, b_ih_r, b_hh_r,
           W_out, b_out, depth):
    global LAST_EXEC_NS, LAST_RESULTS
    encoding = np.asarray(encoding, np.float32)
    W_hh = {"l": np.asarray(W_hh_l, np.float32), "r": np.asarray(W_hh_r, np.float32)}
    b_ih = {"l": np.asarray(b_ih_l, np.float32), "r": np.asarray(b_ih_r, np.float32)}
    b_hh = {"l": np.asarray(b_hh_l, np.float32), "r": np.asarray(b_hh_r, np.float32)}
    W_out = np.asarray(W_out, np.float32)
    b_out = np.asarray(b_out, np.float32)
    depth = int(depth)

    B, H = encoding.shape
    V = W_out.shape[0]
    tok = (B // N_CORES) * (1 << depth) if B % N_CORES == 0 else 0
    if (depth < 1 or B % N_CORES or H % P or P % (B // N_CORES)
            or (tok % P != 0 and tok > P)):
        return _numpy_reference(encoding, W_hh["l"], b_ih["l"], b_hh["l"],
                                W_hh["r"], b_ih["r"], b_hh["r"],
                                W_out, b_out, depth).astype(np.float32)

    Bc = B // N_CORES
    KH = H // P
    bf16 = ml_dtypes.bfloat16

    nc = _get_compiled(Bc, H, V, depth)

    woutt = np.ascontiguousarray(W_out.T.astype(bf16)).reshape(KH, P, V)
    bout_b = np.ascontiguousarray(
        np.broadcast_to(b_out.astype(bf16)[None, :], (P, V)))
    shared = {"woutt": woutt, "bout": bout_b}
    for s in "lr":
        shared[f"whht_{s}"] = np.ascontiguousarray(
            W_hh[s].T.astype(bf16)).reshape(KH, P, 3 * H)
        shared[f"bias_{s}"] = _packed_bias(b_ih[s], b_hh[s], H, KH)

    encT = encoding.T  # [H, B]
    in_maps = []
    for c in range(N_CORES):
        enc_c = np.ascontiguousarray(encT[:, c * Bc:(c + 1) * Bc]).reshape(KH, P, Bc)
        in_maps.append({"enc_t": enc_c, **shared})

    from concourse import bass_utils
    kw = {}
    if TRACE:
        kw["tmpdir"] = os.environ.get("BASS_TRACE_DIR") or None
    res = bass_utils.run_bass_kernel_spmd(
        nc, in_maps, core_ids=list(range(N_CORES)), trace=TRACE, **kw)
    LAST_EXEC_NS = res.exec_time_ns
    LAST_RESULTS = res
    out = np.concatenate([r["out"] for r in res.results], axis=0)
    return np.ascontiguousarray(out.astype(np.float32))


# revision 36
# speedup vs baseline: 1.0293x; 1.0293x over previous
"""DecoderTreeRNN Trainium2 kernel.

Computes: h0 = relu(encoding); expand a depth-`depth` binary tree with two
zero-input GRU cells (left/right); project every leaf hidden state with W_out
and take log_softmax over the vocab.

Strategy: pure data parallel over 8 NeuronCores (batch sharded), GRU weights
and the output projection replicated.  On-core layout is transposed
([hidden-chunk on partitions, tokens on the free dim]) so all matmuls
contract over partitions and the softmax reduction runs along the free dim.
"""

import os
import sys
from contextlib import ExitStack

import numpy as np

for _p in ("/opt/trn_rl_repo", "/root/.axon_site/_ro/trn_rl_repo"):
    if os.path.isdir(_p) and _p not in sys.path:
        sys.path.insert(0, _p)

import ml_dtypes

N_CORES = 8
P = 128
TTILE = 512  # token tile for GRU matmuls (max fp32 moving free dim)
NBF = 512  # fp32 elements per PSUM bank
VGW = 4 * NBF  # vocab group width (4 PSUM banks)
SPLIT_FRAC = 0.45  # fraction of final log_softmax subtract done on ScalarE

# Set by test harness to capture a profile on the next kernel() call.
TRACE = False
# CoreSim cannot interpret strided-partition DMA views; test_sim sets this
# to fall back to dense per-leaf stores (HW always uses the strided path).
SIM_SAFE_DMA = False
LAST_EXEC_NS = None
LAST_RESULTS = None

_COMPILE_CACHE = {}


def _bitrev(x, bits):
    r = 0
    for _ in range(bits):
        r = (r << 1) | (x & 1)
        x >>= 1
    return r


def _numpy_reference(encoding, W_hh_l, b_ih_l, b_hh_l, W_hh_r, b_ih_r, b_hh_r,
                     W_out, b_out, depth):
    def gru(h, W, b_ih, b_hh):
        Hd = h.shape[-1]
        gh = h @ W.T + b_hh
        r = 1.0 / (1.0 + np.exp(-(b_ih[:Hd] + gh[..., :Hd])))
        z = 1.0 / (1.0 + np.exp(-(b_ih[Hd:2 * Hd] + gh[..., Hd:2 * Hd])))
        n = np.tanh(b_ih[2 * Hd:] + r * gh[..., 2 * Hd:])
        return (1.0 - z) * n + z * h

    h = np.maximum(encoding, 0.0)[:, None, :]
    for _ in range(depth):
        left = gru(h, W_hh_l, b_ih_l, b_hh_l)
        right = gru(h, W_hh_r, b_ih_r, b_hh_r)
        h = np.stack([left, right], axis=2).reshape(h.shape[0], -1, h.shape[-1])
    logits = h @ W_out.T + b_out
    m = logits.max(axis=-1, keepdims=True)
    e = np.exp(logits - m)
    return (logits - m) - np.log(e.sum(axis=-1, keepdims=True))


def _patch_act_tables(bacc, mybir):
    """Constrain the ACT table-set chooser so the GRU phase and the
    projection phase each stick to ONE set (2 loads total instead of 2
    per token chunk).  Only the chooser's view is filtered; the runtime
    tables are the real (full) sets, so execution is unchanged."""
    from concourse import hw_specs
    AF = mybir.ActivationFunctionType
    orig = hw_specs.get_activation_tables
    if getattr(bacc.get_activation_tables, "_treernn_patch", False):
        return
    keep = {
        "sigmoid_and_others": {AF.Sigmoid, AF.Tanh, AF.Relu},
        "natural_log_exp_and_others": {AF.Exp, AF.Ln, AF.Identity, AF.Copy},
    }
    controlled = set().union(*keep.values())

    def patched(arch):
        tabs = {k: set(v) for k, v in orig(arch).items()}
        for name, s in tabs.items():
            s -= controlled
            s |= keep.get(name, set())
        return tabs

    patched._treernn_patch = True
    bacc.get_activation_tables = patched


def _build(Bc, H, V, depth):
    """Build + compile the single-core SPMD program (identical on all cores)."""
    import concourse.bass as bass  # noqa: F401
    import concourse.tile as tile
    from concourse import bacc, mybir

    f32 = mybir.dt.float32
    bf16 = mybir.dt.bfloat16
    AF = mybir.ActivationFunctionType
    OP = mybir.AluOpType
    _patch_act_tables(bacc, mybir)

    KH = H // P
    H3 = 3 * H
    L = 1 << depth
    TOK = Bc * L
    NTC = (TOK + P - 1) // P
    # vocab tiles (one PSUM bank each) and groups of up to 4 tiles
    vts = [NBF] * (V // NBF) + ([V % NBF] if V % NBF else [])
    vgroups = []
    pos = 0
    while pos < V:
        w = min(VGW, V - pos)
        vgroups.append((pos, w))
        pos += w
    NVG = len(vgroups)
    split = min(V, max(0, int(V * SPLIT_FRAC)) & ~15)

    nc = bacc.Bacc("TRN2", target_bir_lowering=False, debug=False,
                   num_devices=N_CORES)

    enc_d = nc.dram_tensor("enc_t", [KH, P, Bc], f32, kind="ExternalInput").ap()
    whh_d = {s: nc.dram_tensor(f"whht_{s}", [KH, P, H3], bf16,
                               kind="ExternalInput").ap() for s in "lr"}
    # packed per-side biases: cols [0:2K]=sigmoid(r,z), [2K:3K]=tanh, [3K:4K]=n_hh
    bias_d = {s: nc.dram_tensor(f"bias_{s}", [P, 4 * KH], f32,
                                kind="ExternalInput").ap() for s in "lr"}
    wout_d = nc.dram_tensor("woutt", [KH, P, V], bf16, kind="ExternalInput").ap()
    bout_d = nc.dram_tensor("bout", [P, V], bf16, kind="ExternalInput").ap()
    out_d = nc.dram_tensor("out", [Bc, L, V], f32, kind="ExternalOutput").ap()

    # all vocab-group weight tiles are staged before the GRU phase: their
    # DMA (~10 MB) overlaps GRU compute and fits SBUF since the GRU state
    # went bf16
    n_wv_early = NVG

    with tile.TileContext(nc) as tc, ExitStack() as ctx:
        constp = ctx.enter_context(tc.tile_pool(name="const", bufs=1))
        ht2p = ctx.enter_context(tc.tile_pool(name="ht2", bufs=1))
        ht2 = ht2p.tile([P, KH, TOK], bf16)
        wvep = ctx.enter_context(tc.tile_pool(name="wout_early", bufs=1))
        bop = ctx.enter_context(tc.tile_pool(name="bout", bufs=1))

        bsig, btanh, bnhh = {}, {}, {}
        for s in "lr":
            bt = constp.tile([P, 4 * KH], f32, name=f"bias{s}")
            nc.sync.dma_start(out=bt, in_=bias_d[s])
            bsig[s] = bt[:, :2 * KH]
            btanh[s] = bt[:, 2 * KH:3 * KH]
            bnhh[s] = bt[:, 3 * KH:]

        def load_proj_consts():
            # issued AFTER the GRU weights so they don't head-block the
            # sync-engine DMA queue and delay the GRU start
            bout_sb = bop.tile([P, V], bf16)
            nc.sync.dma_start(out=bout_sb, in_=bout_d)
            wv = []
            for vg, (vs, vw) in enumerate(vgroups[:n_wv_early]):
                wt = wvep.tile([P, KH, vw], bf16, name=f"wv{vg}")
                for k in range(KH):
                    nc.sync.dma_start(out=wt[:, k, :],
                                      in_=wout_d[k, :, vs:vs + vw])
                wv.append(wt)
            return bout_sb, wv

        # ---------------- GRU tree expansion (fp32) ----------------
        with tc.tile_pool(name="gwhh", bufs=1) as gwp, \
             tc.tile_pool(name="gh", bufs=1) as ghp, \
             tc.tile_pool(name="gact", bufs=2) as gap, \
             tc.tile_pool(name="gactd", bufs=1) as gdp, \
             tc.tile_pool(name="gpsum", bufs=2, space="PSUM") as gpp:
            whh = {}
            for s in "lr":
                w = gwp.tile([P, KH, H3], bf16, name=f"whh{s}")
                for k in range(KH):
                    nc.sync.dma_start(out=w[:, k, :], in_=whh_d[s][k])
                whh[s] = w

            enc_sb = gdp.tile([P, KH, Bc], f32, name="enc_stage")
            for k in range(KH):
                nc.sync.dma_start(out=enc_sb[:, k, :], in_=enc_d[k])
            h_cur = ghp.tile([P, KH, Bc], bf16, name="h_l0")
            nc.scalar.activation(out=h_cur, in_=enc_sb, func=AF.Relu)

            bout_sb, wv = load_proj_consts()

            for lvl in range(depth):
                t = Bc << lvl
                h_nxt = ghp.tile([P, KH, 2 * t], bf16, name=f"h_l{lvl + 1}")
                for si, s in enumerate("lr"):
                    soff = si * t
                    for t0 in range(0, t, TTILE):
                        tt = min(TTILE, t - t0)
                        hs = h_cur[:, :, t0:t0 + tt]
                        r_sb = gap.tile([P, KH, TTILE], bf16, name="g_r")[:, :, :tt]
                        z_sb = gap.tile([P, KH, TTILE], bf16, name="g_z")[:, :, :tt]
                        n_sb = gap.tile([P, KH, TTILE], bf16, name="g_n")[:, :, :tt]
                        d_sb = gdp.tile([P, KH, TTILE], bf16, name="g_d")[:, :, :tt]
                        for gi in range(3):  # r, z, n
                            ps = gpp.tile([P, KH, TTILE], f32,
                                          name="g_ps")[:, :, :tt]
                            for gc in range(KH):
                                col = gi * H + gc * P
                                for k in range(KH):
                                    nc.tensor.matmul(
                                        ps[:, gc, :],
                                        lhsT=whh[s][:, k, col:col + P],
                                        rhs=hs[:, k, :],
                                        start=(k == 0), stop=(k == KH - 1))
                            if gi == 0:
                                for gc in range(KH):
                                    nc.scalar.activation(
                                        out=r_sb[:, gc, :], in_=ps[:, gc, :],
                                        func=AF.Sigmoid,
                                        bias=bsig[s][:, gc:gc + 1])
                            elif gi == 1:
                                for gc in range(KH):
                                    nc.scalar.activation(
                                        out=z_sb[:, gc, :], in_=ps[:, gc, :],
                                        func=AF.Sigmoid,
                                        bias=bsig[s][:, KH + gc:KH + gc + 1])
                            else:
                                for gc in range(KH):
                                    # n_pre = (gh_n + b_hh_n) * r
                                    nc.vector.scalar_tensor_tensor(
                                        out=n_sb[:, gc, :], in0=ps[:, gc, :],
                                        scalar=bnhh[s][:, gc:gc + 1],
                                        in1=r_sb[:, gc, :],
                                        op0=OP.add, op1=OP.mult)
                                for gc in range(KH):
                                    nc.scalar.activation(
                                        out=n_sb[:, gc, :], in_=n_sb[:, gc, :],
                                        func=AF.Tanh,
                                        bias=btanh[s][:, gc:gc + 1])
                        # h' = n + z * (h - n)
                        nc.vector.tensor_tensor(d_sb, hs, n_sb, OP.subtract)
                        nc.vector.tensor_tensor(d_sb, d_sb, z_sb, OP.mult)
                        nc.vector.tensor_tensor(
                            h_nxt[:, :, soff + t0:soff + t0 + tt],
                            d_sb, n_sb, OP.add)
                h_cur = h_nxt

            # in-order leaf layout: node blocks were built choice-LSB-first,
            # reference wants choice-MSB-first -> bit-reversal permute.
            # Within each 128-token chunk, tokens are INTERLEAVED as
            # p = b*NLL + ll so a per-leaf output DMA reads stride-NLL
            # partitions (touches all 16 SBUF port groups, not 4).
            NLL = max(1, min(P // Bc, L))  # leaves per token chunk
            for node in range(L):
                leaf = _bitrev(node, depth)
                tcp, ll = leaf // NLL, leaf % NLL
                base = tcp * P + ll
                nc.vector.tensor_copy(
                    out=ht2[:, :, base:base + (Bc - 1) * NLL + 1:NLL],
                    in_=h_cur[:, :, node * Bc:(node + 1) * Bc])

        # ---------------- output projection + log_softmax ----------------
        with tc.tile_pool(name="wout_late", bufs=1) as wvp, \
             tc.tile_pool(name="ypool", bufs=2) as yp, \
             tc.tile_pool(name="stat", bufs=2) as stp, \
             tc.tile_pool(name="escratch", bufs=2) as esp, \
             tc.tile_pool(name="ppsum", bufs=2, space="PSUM") as ppp:
            for vg, (vs, vw) in enumerate(vgroups[n_wv_early:], n_wv_early):
                wt = wvp.tile([P, KH, vw], bf16, name=f"wv{vg}")
                for k in range(KH):
                    nc.sync.dma_start(out=wt[:, k, :], in_=wout_d[k, :, vs:vs + vw])
                wv.append(wt)

            def out_dma(tci, pc, y, v0, v1):
                """Per-leaf stores; partitions are interleaved (p = b*NLL+ll)
                so each DMA's 32 source partitions stride across all 16 SBUF
                port groups and its rows spread over all 16 SDMA engines."""
                nll = pc // Bc
                if SIM_SAFE_DMA:
                    # dense-partition source; dst dims follow p = b*nll + ll
                    dst = out_d.rearrange("b (lc ll) v -> lc b ll v", ll=nll)
                    nc.sync.dma_start(out=dst[tci][:, :, v0:v1],
                                      in_=y[:, v0:v1])
                    return
                for ll in range(nll):
                    leaf = tci * nll + ll
                    nc.sync.dma_start(out=out_d[:, leaf, v0:v1],
                                      in_=y[ll:pc:nll, v0:v1])

            import bass_rust as _br

            def emit_tail(st, after_tt=None):
                """Softmax tail for a finished chunk: c = ln(sum), out -= c."""
                tci, pc, y, sums, cs = st
                nc.vector.tensor_reduce(out=cs[:, 0:1], in_=sums,
                                        axis=mybir.AxisListType.X, op=OP.add)
                nc.scalar.activation(out=cs[:, 0:1], in_=cs[:, 0:1],
                                     func=AF.Ln)
                nc.vector.tensor_scalar(out=cs[:, 1:2], in0=cs[:, 0:1],
                                        scalar1=-1.0, scalar2=None,
                                        op0=OP.mult)
                if split > 0:
                    nc.scalar.activation(out=y[:, :split], in_=y[:, :split],
                                         func=AF.Identity, bias=cs[:, 1:2])
                    out_dma(tci, pc, y, 0, split)
                if split < V:
                    # Must NOT precede the next chunk's first bias-add in the
                    # VectorE FIFO (that stalls the PSUM-slot recycle), so
                    # order it explicitly after that bias-add.
                    sub = nc.vector.tensor_scalar(out=y[:, split:],
                                                  in0=y[:, split:],
                                                  scalar1=cs[:, 0:1],
                                                  scalar2=None,
                                                  op0=OP.subtract)
                    if after_tt is not None:
                        _br.add_dep_helper(
                            sub.ins, after_tt.ins, sync=False,
                            reason="tail subtract yields to next bias-add")
                    out_dma(tci, pc, y, split, V)

            pending = None  # previous chunk's tail, pipelined one chunk late
            for tci in range(NTC):
                pc = min(P, TOK - tci * P)  # tokens in this chunk
                y = yp.tile([P, V], f32, name="y")[:pc]
                sums = stp.tile([P, NVG], f32, name="sums")[:pc]
                cs = stp.tile([P, 2], f32, name="cs")[:pc]
                for vg, (vs, vw) in enumerate(vgroups):
                    ps = ppp.tile([P, VGW], f32, name="p_vg")[:pc, :vw]
                    for vt0 in range(0, vw, NBF):
                        w = min(NBF, vw - vt0)
                        pslice = ps[:, vt0:vt0 + w]
                        for k in range(KH):
                            nc.tensor.matmul(
                                pslice,
                                lhsT=ht2[:, k, tci * P:tci * P + pc],
                                rhs=wv[vg][:, k, vt0:vt0 + w],
                                start=(k == 0), stop=(k == KH - 1))
                    tt = nc.vector.tensor_tensor(
                        y[:, vs:vs + vw], ps, bout_sb[:pc, vs:vs + vw], OP.add)
                    if vg == 0:
                        first_tt = tt
                    # exp main output is discarded (into SBUF scratch, NOT the
                    # PSUM tile — writing PSUM would extend the slot lifetime
                    # and stall the matmuls two vocab-groups ahead);
                    # only the per-partition running sum matters
                    esc = esp.tile([P, VGW], bf16, name="e_sc")[:pc, :vw]
                    nc.scalar.activation(out=esc, in_=y[:, vs:vs + vw],
                                         func=AF.Exp,
                                         accum_out=sums[:, vg:vg + 1])
                    # previous chunk's softmax tail, issued after this
                    # chunk's first vocab-group so its DMA starts early but
                    # doesn't head-block this chunk's drain chain
                    if vg == 0 and pending is not None:
                        emit_tail(pending, after_tt=first_tt)
                pending = (tci, pc, y, sums, cs)
            emit_tail(pending)

    nc.compile()
    return nc


def _packed_bias(b_ih, b_hh, H, KH):
    """[P, 4*KH]: sigmoid biases (b_ih+b_hh for r,z), tanh bias (b_ih_n),
    and the pre-multiply n-gate bias (b_hh_n), per 128-row chunk."""
    P = 128
    sig = (b_ih + b_hh)[:2 * H].reshape(2 * KH, P).T
    tanh = b_ih[2 * H:].reshape(KH, P).T
    nhh = b_hh[2 * H:].reshape(KH, P).T
    return np.ascontiguousarray(np.concatenate([sig, tanh, nhh], axis=1))


def _get_compiled(Bc, H, V, depth):
    key = (Bc, H, V, depth)
    if key not in _COMPILE_CACHE:
        _COMPILE_CACHE[key] = _build(Bc, H, V, depth)
    return _COMPILE_CACHE[key]


def kernel(encoding, W_hh_l, b_ih_l, b_hh_l, W_hh_r, b_ih_r, b_hh_r,
           W_out, b_out, depth):
    global LAST_EXEC_NS, LAST_RESULTS
    encoding = np.asarray(encoding, np.float32)
    W_hh = {"l": np.asarray(W_hh_l, np.float32), "r": np.asarray(W_hh_r, np.float32)}
    b_ih = {"l": np.asarray(b_ih_l, np.float32), "r": np.asarray(b_ih_r, np.float32)}
    b_hh = {"l": np.asarray(b_hh_l, np.float32), "r": np.asarray(b_hh_r, np.float32)}
    W_out = np.asarray(W_out, np.float32)
    b_out = np.asarray(b_out, np.float32)
    depth = int(depth)

    B, H = encoding.shape
    V = W_out.shape[0]
    tok = (B // N_CORES) * (1 << depth) if B % N_CORES == 0 else 0
    if (depth < 1 or B % N_CORES or H % P or P % (B // N_CORES)
            or (tok % P != 0 and tok > P)):
        return _numpy_reference(encoding, W_hh["l"], b_ih["l"], b_hh["l"],
                                W_hh["r"], b_ih["r"], b_hh["r"],
                                W_out, b_out, depth).astype(np.float32)

    Bc = B // N_CORES
    KH = H // P
    bf16 = ml_dtypes.bfloat16

    nc = _get_compiled(Bc, H, V, depth)

    woutt = np.ascontiguousarray(W_out.T.astype(bf16)).reshape(KH, P, V)
    bout_b = np.ascontiguousarray(
        np.broadcast_to(b_out.astype(bf16)[None, :], (P, V)))
    shared = {"woutt": woutt, "bout": bout_b}
    for s in "lr":
        shared[f"whht_{s}"] = np.ascontiguousarray(
            W_hh[s].T.astype(bf16)).reshape(KH, P, 3 * H)
        shared[f"bias_{s}"] = _packed_bias(b_ih[s], b_hh[s], H, KH)

    encT = encoding.T  # [H, B]
    in_maps = []
    for c in range(N_CORES):
        enc_c = np.ascontiguousarray(encT[:, c * Bc:(c + 1) * Bc]).reshape(KH, P, Bc)
        in_maps.append({"enc_t": enc_c, **shared})

    from concourse import bass_utils
    kw = {}
    if TRACE:
        kw["tmpdir"] = os.environ.get("BASS_TRACE_DIR") or None
    res = bass_utils.run_bass_kernel_spmd(
        nc, in_maps, core_ids=list(range(N_CORES)), trace=TRACE, **kw)
    LAST_EXEC_NS = res.exec_time_ns
    LAST_RESULTS = res
    out = np.concatenate([r["out"] for r in res.results], axis=0)
    return np.ascontiguousarray(out.astype(np.float32))
